# revision 40
# baseline (speedup 1.0000x reference)
"""3-layer GCN (DGL GraphConv, norm='both') on 8 Trainium2 NeuronCores.

v6: L0 push-down projection + descriptor-minimized SPMD single-NEFF design.
  - L0 gathers RAW ns-prescaled features (512B rows, bf16) from four
    pre-staged replicated DRAM tables (global node quarters, int16-indexable)
    so gathers saturate the (serialized) Q7 SWDGE pipeline from t~=0 with no
    projection/AllGather gating. Aggregation runs at 256 feats (two matmul
    halves sharing each one-hot), then W1-projection + relu per dst window.
  - L1/L2: project own nodes on PE (bf16), quarter-AllGathers of the
    projected shard, per-edge SWDGE dma_gather (<=1024 idxs/call - HW limit),
    one-hot matmul segment-sum over 128-dst windows.
  - Per layer the in-degree norm is deferred into the next projection's
    per-node scale (nsd = ns*nd); L0's table carries ns only.
  - The last quarter-AG of layers 1-2 is deferred into the next layer's
    first super-group (fired just before its first chunk-3 gather call) so a
    pending AG trigger never head-blocks ready gather work on gpsimd.
  - Host (numpy) does index-only prep + input layout (bf16 cast, transpose,
    ns prescale) - SPMD needs identical instruction streams on all 8 cores.
"""

import numpy as np
import ml_dtypes

import concourse.bacc as bacc
import concourse.bass as bass
import concourse.mybir as mybir
import concourse.tile as tile
from concourse.bass import AP
from concourse.bass_utils import run_bass_kernel_spmd

BF16 = ml_dtypes.bfloat16
F32 = np.float32

N_NODES = 100000
D_IN, D_H1, D_H2, D_OUT = 256, 128, 128, 64
NCORE = 8
NP = N_NODES // NCORE        # 12500 nodes per core
NWIN = (NP + 127) // 128     # 98 windows (last holds 84)
NPPAD = NWIN * 128           # 12544
NCHUNK = 4
# L1/L2 src chunks (core-quartered AllGather tables): the LAST quarter-AG
# gates each layer boundary, keep it small (measured better than packing
# groups under k*1024). max sloc = 8*4064-1 < 32768.
QROWS_L = [4064, 4064, 3328, NP - 11456]
QCUM = [0]
for _q in QROWS_L:
    QCUM.append(QCUM[-1] + _q)
CHUNK_L = [q * NCORE for q in QROWS_L]           # gather chunk sizes
# L0 src chunks (global node quarters of the static feature table); sized so
# most (super0, chunk) groups fit one <=1024-idx call. max sloc0 = 32766.
Q0ROWS = [22411, 22411, 22411, 32767]
Q0CUM = [0]
for _q in Q0ROWS:
    Q0CUM.append(Q0CUM[-1] + _q)
GCALL = 1024                 # idxs per dma_gather call (hard HW limit)
SUPW = 4                     # windows per gather super-group (L1/L2)
SUPW0 = 2                    # windows per super-group (L0; 512B rows => 2x
#                              SBUF per tile, so half the windows)
NQUEUE = 4                   # SWDGE queues
# last proj window needed by quarter-AG q (L1/L2)
AGWIN = [((QCUM[q + 1] + 127) // 128) - 1 for q in range(NCHUNK)]


def _mk_sched(core, win, sloc, chunk, dcol, supw, zero_virgin_tails=False):
    """Static gather/aggregation schedule for one edge->(cell) bucketing.

    cells (dst-window, src-chunk) sized to the cross-core max count and
    concatenated per (super-group, chunk) with only the group tail padded
    to a 128-slot tile. Tiles may span window boundaries; each
    (window, chunk, tile) matmul gets its own one-hot column block with -1
    (no match) marking rows of other windows.
    """
    ncell = NCORE * NWIN * NCHUNK
    cell = ((core * NWIN + win) * NCHUNK + chunk).astype(np.int64)
    order = np.argsort(cell * 32768 + sloc, kind="stable")
    counts = np.bincount(cell, minlength=ncell).reshape(NCORE, NWIN, NCHUNK)
    m_wj = counts.max(axis=0).astype(np.int64)      # uniform cell sizes

    starts = np.zeros(ncell + 1, np.int64)
    np.cumsum(np.bincount(cell, minlength=ncell), out=starts[1:])

    nsup = (NWIN + supw - 1) // supw
    sup_ws = [list(range(s * supw, min((s + 1) * supw, NWIN)))
              for s in range(nsup)]

    A_wj = np.zeros((NWIN, NCHUNK), np.int64)   # absolute slot offset of cell
    gtile0 = {}
    gtiles = {}
    sup_tile0 = []
    p = 0
    for s in range(nsup):
        sup_tile0.append(p // 128)
        for j in range(NCHUNK):
            gtile0[(s, j)] = p // 128
            g0 = p
            for w in sup_ws[s]:
                A_wj[w, j] = p
                p += int(m_wj[w, j])
            p = ((p + 127) // 128) * 128
            gtiles[(s, j)] = (p - g0) // 128
    NT2 = p // 128
    sup_tile0.append(NT2)

    mm_of_sup = []
    nmm_w = np.zeros(NWIN, np.int64)
    for s in range(nsup):
        lst = []
        for w in sup_ws[s]:
            for j in range(NCHUNK):
                if m_wj[w, j] == 0:
                    continue
                a, m = int(A_wj[w, j]), int(m_wj[w, j])
                for t in range(a // 128, (a + m + 127) // 128):
                    lst.append((w, j, t))
                    nmm_w[w] += 1
        mm_of_sup.append(lst)
    NMM = int(sum(len(x) for x in mm_of_sup))
    max_sup_mm = max(len(x) for x in mm_of_sup)
    max_sup_tiles = max(sup_tile0[s + 1] - sup_tile0[s] for s in range(nsup))

    sidx = np.full((NCORE, NT2 * 128), -1, np.int16)
    dcol_slot = np.full((NCORE, NT2 * 128), -1, np.int64)
    for c in range(NCORE):
        for w in range(NWIN):
            for j in range(NCHUNK):
                cid = (c * NWIN + w) * NCHUNK + j
                e = order[starts[cid]:starts[cid + 1]]
                n = len(e)
                a = int(A_wj[w, j])
                sidx[c, a:a + n] = sloc[e]
                sidx[c, a + n:a + int(m_wj[w, j])] = 0   # interior pad
                dcol_slot[c, a:a + n] = dcol[e]
    # group tails keep idx=-1 (stripped at call end by the Q7) EXCEPT for
    # the first buffer rotation of the pool that first touches virgin SBUF:
    # there the tails gather row 0 (finite data) so no memset is needed to
    # keep stale-NaN out of the one-hot-zero multiplies
    if zero_virgin_tails:
        for s in range(min(3, nsup)):
            for j in range(NCHUNK):
                a0 = gtile0[(s, j)] * 128
                sz = gtiles[(s, j)] * 128
                seg = sidx[:, a0:a0 + sz]
                np.maximum(seg, 0, out=seg)

    dcol_mm = np.full((NCORE, NMM, 128), -1.0, F32)
    mm_idx = 0
    for s in range(nsup):
        for (w, j, t) in mm_of_sup[s]:
            a, m = int(A_wj[w, j]), int(m_wj[w, j])
            lo = max(a, t * 128)
            hi = min(a + m, (t + 1) * 128)
            for c in range(NCORE):
                seg = dcol_slot[c, lo:hi]
                dst_rows = np.arange(lo - t * 128, hi - t * 128)
                valid = seg >= 0
                dcol_mm[c, mm_idx, dst_rows[valid]] = seg[valid]
            mm_idx += 1
    assert mm_idx == NMM

    calls = []   # (chunk j, abs slot offset, n_idxs, super)
    for s in range(nsup):
        for j in range(NCHUNK):
            t0 = gtile0[(s, j)]
            off = t0 * 128
            greal = int(sum(m_wj[w, j] for w in sup_ws[s]))
            if zero_virgin_tails and s < 3:
                greal = gtiles[(s, j)] * 128   # cover tail slots too
            q = 0
            while q < greal:
                n = min(GCALL, ((greal - q + 15) // 16) * 16)
                calls.append((j, off + q, n, s))
                q += n

    def idx_layout(a):      # [n] int16 -> [128, n//16]
        return np.tile(a.reshape(-1, 16).T, (8, 1))

    sidx_l = np.stack([idx_layout(sidx[c]) for c in range(NCORE)])
    dcol_l = np.ascontiguousarray(
        dcol_mm.transpose(0, 2, 1)).astype(BF16)    # [NCORE, 128, NMM]

    return dict(NT2=NT2, NMM=NMM, calls=calls, mm_of_sup=mm_of_sup,
                nmm_w=nmm_w, sup_tile0=sup_tile0, nsup=nsup, sup_ws=sup_ws,
                max_sup_tiles=max_sup_tiles, max_sup_mm=max_sup_mm,
                sidx=sidx_l, dcol=dcol_l)


def _host_prep(feat, W1, b1, W2, b2, W3, b3, src, dst):
    src = np.asarray(src).astype(np.int64)
    dst = np.asarray(dst).astype(np.int64)
    for b in (b1, b2, b3):
        assert np.max(np.abs(np.asarray(b))) == 0.0, \
            "nonzero bias needs the undeferred-nd path"

    deg_out = np.bincount(src, minlength=N_NODES).astype(F32)
    deg_in = np.bincount(dst, minlength=N_NODES).astype(F32)
    ns = 1.0 / np.sqrt(np.maximum(deg_out, 1.0))
    nd = 1.0 / np.sqrt(np.maximum(deg_in, 1.0))
    nsd = ns * nd

    core = dst // NP
    dloc = dst % NP
    win = dloc // 128
    dcol = (dloc % 128).astype(np.int32)

    # L1/L2 bucketing: src by core-quarter of the AllGather table
    c_src = src // NP
    r_src = src % NP
    chunk = np.searchsorted(np.array(QCUM[1:]), r_src, side="right")
    qr = np.array([QROWS_L[q] for q in chunk])
    q0 = np.array([QCUM[q] for q in chunk])
    sloc = (c_src * qr + (r_src - q0)).astype(np.int16)
    s12 = _mk_sched(core, win, sloc, chunk, dcol, SUPW)

    # L0 bucketing: src by global quarter of the static feature table
    chunk0 = np.searchsorted(np.array(Q0CUM[1:]), src, side="right")
    b0 = np.array([Q0CUM[q] for q in chunk0])
    sloc0 = (src - b0).astype(np.int16)
    s0 = _mk_sched(core, win, sloc0, chunk0, dcol, SUPW0,
                   zero_virgin_tails=True)

    # L0 tables: ns-prescaled raw features, bf16, global node order,
    # quartered; replicated to every core.
    feat_pre = (np.asarray(feat).astype(F32) * ns[:, None]).astype(BF16)
    tabs = [np.ascontiguousarray(feat_pre[Q0CUM[j]:Q0CUM[j + 1]])
            for j in range(NCHUNK)]

    nsdp = np.zeros((NCORE, 128, NWIN), F32)
    ndp = np.zeros((NCORE, 128, NWIN), F32)
    for c in range(NCORE):
        for arr, dstp in ((nsd, nsdp), (nd, ndp)):
            v = np.zeros(NPPAD, F32)
            v[:NP] = arr[c * NP:(c + 1) * NP]
            dstp[c] = v.reshape(NWIN, 128).T

    ohchunk = min(32, max(s12["max_sup_mm"], s0["max_sup_mm"]))
    consts = dict(
        w1=np.asarray(W1).astype(F32).astype(BF16),
        w2=np.asarray(W2).astype(F32).astype(BF16),
        w3p=np.pad(np.asarray(W3).astype(F32), ((0, 0), (0, 128 - D_OUT))).astype(BF16),
        iota=np.tile(np.arange(128, dtype=F32).astype(BF16)[None, :],
                     (128, ohchunk)),
        tab0=tabs[0], tab1=tabs[1], tab2=tabs[2], tab3=tabs[3],
    )
    sched = dict(s12=s12, s0=s0, ohchunk=ohchunk)
    percore = dict(nsdp=nsdp, ndp=ndp,
                   sidx=s12["sidx"], dcol=s12["dcol"],
                   sidx0=s0["sidx"], dcol0=s0["dcol"])
    return sched, consts, percore


def _build(sched):
    s12 = sched["s12"]; s0 = sched["s0"]; ohchunk = sched["ohchunk"]
    NT2 = s12["NT2"]; NMM = s12["NMM"]
    NT20 = s0["NT2"]; NMM0 = s0["NMM"]
    nsup = s12["nsup"]; nsup0 = s0["nsup"]
    sup_ws = s12["sup_ws"]; sup_ws0 = s0["sup_ws"]
    # shared gs/oh pools: byte-identical padded slots for both layouts
    mst12 = s12["max_sup_tiles"]; mst0 = s0["max_sup_tiles"]
    GSB = max(mst12 * 256, mst0 * 512)          # bytes per partition
    GS12PAD = (GSB + 255) // 256
    GS0PAD = (GSB + 511) // 512
    msm = max(s12["max_sup_mm"], s0["max_sup_mm"])

    calls_by_sup = {}
    for c in s12["calls"]:
        calls_by_sup.setdefault(c[3], []).append(c)
    calls_by_sup0 = {}
    for c in s0["calls"]:
        calls_by_sup0.setdefault(c[3], []).append(c)

    nc = bacc.Bacc("TRN2", target_bir_lowering=False, debug=False,
                   num_devices=NCORE, num_swdge_queues=NQUEUE)
    dt = mybir.dt

    tab_t = [nc.dram_tensor(f"tab{j}", [Q0ROWS[j], D_IN], dt.bfloat16,
                            kind="ExternalInput") for j in range(NCHUNK)]
    w1_t = nc.dram_tensor("w1", [D_IN, D_H1], dt.bfloat16, kind="ExternalInput")
    w2_t = nc.dram_tensor("w2", [D_H1, D_H2], dt.bfloat16, kind="ExternalInput")
    w3_t = nc.dram_tensor("w3p", [D_H2, 128], dt.bfloat16, kind="ExternalInput")
    nsd_t = nc.dram_tensor("nsdp", [128, NWIN], dt.float32, kind="ExternalInput")
    nd_t = nc.dram_tensor("ndp", [128, NWIN], dt.float32, kind="ExternalInput")
    sidx_t = nc.dram_tensor("sidx", [128, NT2 * 8], dt.int16, kind="ExternalInput")
    dcol_t = nc.dram_tensor("dcol", [128, NMM], dt.bfloat16, kind="ExternalInput")
    sidx0_t = nc.dram_tensor("sidx0", [128, NT20 * 8], dt.int16,
                             kind="ExternalInput")
    dcol0_t = nc.dram_tensor("dcol0", [128, NMM0], dt.bfloat16,
                             kind="ExternalInput")
    iota_t = nc.dram_tensor("iota", [128, ohchunk * 128], dt.bfloat16,
                            kind="ExternalInput")
    out_t = nc.dram_tensor("out", [NP, D_OUT], dt.float32, kind="ExternalOutput")

    qcount = [0]

    def next_queue():
        q = qcount[0] % NQUEUE
        qcount[0] += 1
        return q

    with tile.TileContext(nc) as tc:
        with (
            tc.tile_pool(name="const", bufs=1) as cpool,
            tc.tile_pool(name="hbuf", bufs=1) as hpool,
            tc.tile_pool(name="gb", bufs=3) as gpool,
            tc.tile_pool(name="sx", bufs=3) as sxpool,
            tc.tile_pool(name="work", bufs=3) as wpool,
            tc.tile_pool(name="oh", bufs=3) as ohpool,
            tc.tile_pool(name="ps", bufs=6, space="PSUM") as ppool,
            tc.tile_pool(name="pj", bufs=2, space="PSUM") as pjpool,
            tc.tile_pool(name="dram", bufs=1, space="DRAM") as dpool,
        ):
            w1a_s = cpool.tile([128, D_H1], dt.bfloat16)
            w1b_s = cpool.tile([128, D_H1], dt.bfloat16)
            w2_s = cpool.tile([D_H1, D_H2], dt.bfloat16)
            w3_s = cpool.tile([D_H2, 128], dt.bfloat16)
            nsd_s = cpool.tile([128, NWIN], dt.float32)
            nd_s = cpool.tile([128, NWIN], dt.float32)
            sidx_s = cpool.tile([128, NT2 * 8], dt.int16)
            dcol_s = cpool.tile([128, NMM], dt.bfloat16)
            iota_s = cpool.tile([128, ohchunk * 128], dt.bfloat16)

            # iota/weights/norms on the sync queue (small, needed early);
            # the BIG L1/L2 index consts go on the Activation HWDGE queue so
            # they don't delay the streamed per-super L0 index slices that
            # the sync queue serves during startup
            nc.sync.dma_start(iota_s[:], iota_t.ap())
            nc.sync.dma_start(w1a_s[:], w1_t.ap()[0:128, :])
            nc.sync.dma_start(w1b_s[:], w1_t.ap()[128:256, :])
            nc.sync.dma_start(w2_s[:], w2_t.ap())
            nc.sync.dma_start(w3_s[:], w3_t.ap())
            nc.sync.dma_start(nsd_s[:], nsd_t.ap())
            nc.sync.dma_start(nd_s[:], nd_t.ap())
            nc.scalar.dma_start(sidx_s[:], sidx_t.ap())
            nc.scalar.dma_start(dcol_s[:], dcol_t.ap())

            h_s = hpool.tile([128, NWIN * 128], dt.bfloat16)   # hT (feat x nodes)

            tins = [dpool.tile([NP, 128], dt.bfloat16, name=f"tin{L}")
                    for L in (1, 2)]
            tfulls = [[dpool.tile([CHUNK_L[q], 128], dt.bfloat16,
                                  name=f"tfull{L}_{q}", addr_space="Shared")
                       for q in range(NCHUNK)] for L in (1, 2)]

            # zero the gather buffers once (stale bytes multiply with
            # one-hot zeros, so they must be finite). The first rotation
            # (L0 supers 0-2, zero_virgin_tails) gathers its full padded
            # extent, so only the bytes BEYOND each super's coverage need
            # zeroing — a few tiles instead of ~10us each on DVE ahead of
            # the first one-hot builds.
            for b in range(3):
                gz = gpool.tile([128, GS12PAD, 128], dt.bfloat16,
                                name="gsz", tag="gs",
                                padded_shape=[128, GS12PAD, 128])
                cov_tiles = 0
                if b < nsup0:
                    st = s0["sup_tile0"][b + 1] - s0["sup_tile0"][b]
                    cov_tiles = st * 2          # 512B gs0 tiles in 256B units
                if cov_tiles < GS12PAD:
                    nc.vector.memset(gz[:, cov_tiles:GS12PAD, :], 0.0)

            def proj4(L, w0, wn):
                # wn (<=4) projection windows into one PSUM bank, one wide
                # scale instr, per-window tin writes. L in {1, 2}; the
                # in-degree norm of layer L-1 is folded into nsd.
                ppj = pjpool.tile([128, wn * 128], dt.float32, name=f"pj{L}",
                                  tag="pj", padded_shape=[128, 512])
                for k in range(wn):
                    w = w0 + k
                    osl = ppj[:, k * 128:(k + 1) * 128]
                    rhs = w2_s if L == 1 else w3_s
                    nc.tensor.matmul(osl,
                                     lhsT=h_s[:, w * 128:(w + 1) * 128],
                                     rhs=rhs[:], start=True, stop=True)
                ssl = nsd_s[:, w0:w0 + wn]
                sbc = AP(ssl.tensor, ssl.offset, list(ssl.ap) + [[0, 128]])
                pbf = wpool.tile([128, wn * 128], dt.bfloat16, name="pbf",
                                 tag="pbf", padded_shape=[128, 512])
                nc.vector.tensor_tensor(out=pbf[:, 0:wn * 128],
                                        in0=ppj[:, 0:wn * 128], in1=sbc,
                                        op=mybir.AluOpType.mult)
                for k in range(wn):
                    w = w0 + k
                    wsz = min(128, NP - w * 128)
                    nc.sync.dma_start(tins[L - 1][w * 128:w * 128 + wsz, :],
                                      pbf[:wsz, k * 128:(k + 1) * 128])

            def ag(L, q):
                nc.gpsimd.collective_compute(
                    "AllGather", mybir.AluOpType.bypass,
                    replica_groups=[list(range(NCORE))],
                    ins=[tins[L - 1][QCUM[q]:QCUM[q + 1], :].opt()],
                    outs=[tfulls[L - 1][q][:].opt()],
                )

            def build_oh(dsl_src, mm0, nmm_s, tagsfx=""):
                oh = ohpool.tile([128, nmm_s * 128], dt.bfloat16,
                                 name=f"oh{tagsfx}", tag="oh",
                                 padded_shape=[128, msm * 128])
                q = 0
                while q < nmm_s:
                    nb = min(ohchunk, nmm_s - q)
                    dsl = dsl_src(q, nb)
                    bcast = AP(dsl.tensor, dsl.offset,
                               list(dsl.ap) + [[0, 128]])
                    nc.vector.tensor_tensor(
                        out=oh[:, q * 128:(q + nb) * 128],
                        in0=iota_s[:, 0:nb * 128],
                        in1=bcast,
                        op=mybir.AluOpType.is_equal)
                    q += nb
                return oh

            def agg0(s):
                # L0: gather 256-feat raw rows, aggregate both halves via
                # shared one-hots, project through W1 + relu into h_s
                stile0 = s0["sup_tile0"][s]
                stiles = s0["sup_tile0"][s + 1] - stile0
                smms = s0["mm_of_sup"][s]
                nmm_s = len(smms)
                mm0 = sum(len(s0["mm_of_sup"][ss]) for ss in range(s))
                sxs = sxpool.tile([128, stiles * 8], dt.int16, name="sx0",
                                  tag="sx0", padded_shape=[128, mst0 * 8])
                nc.sync.dma_start(
                    sxs[:], sidx0_t.ap()[:, stile0 * 8:(stile0 + stiles) * 8])
                dcs = sxpool.tile([128, nmm_s], dt.bfloat16, name="dc0",
                                  tag="dc0", padded_shape=[128, msm])
                nc.sync.dma_start(dcs[:], dcol0_t.ap()[:, mm0:mm0 + nmm_s])
                gs = gpool.tile([128, stiles, 256], dt.bfloat16,
                                name=f"gs0_{s}", tag="gs",
                                padded_shape=[128, GS0PAD, 256])
                for (j, off, n, cs) in calls_by_sup0.get(s, []):
                    rel = off // 128 - stile0
                    nc.gpsimd.dma_gather(
                        gs[:, rel:rel + (n + 127) // 128, :],
                        tab_d[j][:],
                        sxs[:, (off - stile0 * 128) // 16:
                            (off - stile0 * 128 + n) // 16],
                        n, n, 256,
                        queue_num=next_queue(),
                    )
                oh = build_oh(lambda q, nb: dcs[:, q:q + nb], 0, nmm_s, "0")
                apsA = {}
                apsB = {}
                done = {}
                for mi, (w, j, t) in enumerate(smms):
                    if w not in apsA:
                        apsA[w] = ppool.tile([128, 128], dt.float32,
                                             name="apA", tag="pp")
                        apsB[w] = ppool.tile([128, 128], dt.float32,
                                             name="apB", tag="pp")
                        done[w] = 0
                    k = done[w]
                    ohsl = oh[:, mi * 128:(mi + 1) * 128]
                    first = k == 0
                    last = k == int(s0["nmm_w"][w]) - 1
                    trel = t - stile0
                    nc.tensor.matmul(apsA[w][:], lhsT=gs[:, trel, 0:128],
                                     rhs=ohsl, start=first, stop=last)
                    nc.tensor.matmul(apsB[w][:], lhsT=gs[:, trel, 128:256],
                                     rhs=ohsl, start=first, stop=last)
                    done[w] = k + 1
                    if not last:
                        continue
                    evA = wpool.tile([128, 128], dt.bfloat16, name="evA",
                                     tag="evA")
                    evB = wpool.tile([128, 128], dt.bfloat16, name="evB",
                                     tag="evB")
                    nc.scalar.activation(evA[:], apsA[w][:],
                                         mybir.ActivationFunctionType.Copy)
                    nc.scalar.activation(evB[:], apsB[w][:],
                                         mybir.ActivationFunctionType.Copy)
                    hw = ppool.tile([128, 128], dt.float32, name="hw",
                                    tag="pp")
                    nc.tensor.matmul(hw[:], lhsT=w1a_s[:], rhs=evA[:],
                                     start=True, stop=False)
                    nc.tensor.matmul(hw[:], lhsT=w1b_s[:], rhs=evB[:],
                                     start=False, stop=True)
                    nc.scalar.activation(
                        h_s[:, w * 128:(w + 1) * 128], hw[:],
                        mybir.ActivationFunctionType.Relu)

            def alloc_gs12(L, s):
                stiles = s12["sup_tile0"][s + 1] - s12["sup_tile0"][s]
                return gpool.tile([128, stiles, 128], dt.bfloat16,
                                  name=f"gs{L}_{s}", tag="gs",
                                  padded_shape=[128, GS12PAD, 128])

            def issue12(L, s, gs, only=None, skip=None):
                stile0 = s12["sup_tile0"][s]
                for (j, off, n, cs) in calls_by_sup.get(s, []):
                    if only is not None and j not in only:
                        continue
                    if skip is not None and j in skip:
                        continue
                    rel = off // 128 - stile0
                    nc.gpsimd.dma_gather(
                        gs[:, rel:rel + (n + 127) // 128, :],
                        tfulls[L - 1][j][:],
                        sidx_s[:, off // 16:(off + n) // 16],
                        n, n, 128,
                        queue_num=next_queue(),
                    )

            def agg(L, s, gs):
                # L in {1, 2}: 128-feat aggregation from AllGather tables
                stile0 = s12["sup_tile0"][s]
                smms = s12["mm_of_sup"][s]
                nmm_s = len(smms)
                mm0 = sum(len(s12["mm_of_sup"][ss]) for ss in range(s))
                oh = build_oh(lambda q, nb: dcol_s[:, mm0 + q:mm0 + q + nb],
                              mm0, nmm_s)
                aps_of_w = {}
                done_of_w = {}
                for mi, (w, j, t) in enumerate(smms):
                    if w not in aps_of_w:
                        aps_of_w[w] = ppool.tile([128, 128], dt.float32,
                                                 name=f"ap{L}", tag="pp")
                        done_of_w[w] = 0
                    aps = aps_of_w[w]
                    k = done_of_w[w]
                    ohsl = oh[:, mi * 128:(mi + 1) * 128]
                    first, last = k == 0, k == int(s12["nmm_w"][w]) - 1
                    if L == 1:
                        nc.tensor.matmul(aps[:], lhsT=gs[:, t - stile0, :],
                                         rhs=ohsl, start=first, stop=last)
                    else:
                        nc.tensor.matmul(aps[:, 0:D_OUT], lhsT=ohsl,
                                         rhs=gs[:, t - stile0, 0:D_OUT],
                                         start=first, stop=last)
                    done_of_w[w] = k + 1
                    if not last:
                        continue
                    if L == 1:
                        nc.scalar.activation(
                            h_s[:, w * 128:(w + 1) * 128], aps[:],
                            mybir.ActivationFunctionType.Relu)
                    else:
                        wsz = min(128, NP - w * 128)
                        ob = wpool.tile([128, D_OUT], dt.float32, name="ob",
                                        tag="ob")
                        nc.vector.tensor_scalar(
                            out=ob[:], in0=aps[:, 0:D_OUT],
                            scalar1=nd_s[:, w:w + 1], scalar2=None,
                            op0=mybir.AluOpType.mult)
                        nc.sync.dma_start(
                            out_t.ap()[w * 128:w * 128 + wsz, :], ob[:wsz, :])

            # L0 tables to local DRAM (inputs land in DRAM already; use the
            # dram tensors directly as gather sources)
            tab_d = [tab_t[j].ap() for j in range(NCHUNK)]

            # warmup: a tiny throwaway gather absorbs the one-time Q7
            # ucode/IRAM cold start before the real calls need the engine
            warm_i = cpool.tile([128, 1], dt.int16)
            nc.sync.dma_start(warm_i[:], sidx0_t.ap()[:, 0:1])
            warm = cpool.tile([128, 1, 256], dt.bfloat16)
            nc.gpsimd.dma_gather(warm[:], tab_d[0], warm_i[:], 16, 16, 256,
                                 queue_num=0)

            # ---- L0: gather raw features, aggregate, project; emit L1
            #      projections + quarter-AGs as windows complete ----
            agsup0 = [min(nsup0 - 1, (AGWIN[q] // SUPW0) + 2)
                      for q in range(NCHUNK)]
            nagq = 0
            for s in range(nsup0):
                agg0(s)
                proj4(1, sup_ws0[s][0], len(sup_ws0[s]))
                while nagq < NCHUNK - 1 and s >= agsup0[nagq]:
                    ag(1, nagq)
                    nagq += 1

            # ---- L1/L2 ----
            agsup = [min(nsup - 1, (AGWIN[q] // SUPW) + 2)
                     for q in range(NCHUNK)]
            for L in (1, 2):
                nagq = 0
                NB = 3          # layer-start batch width (= gs pool bufs)
                gs_batch = []
                for s in range(nsup):
                    if s == 0:
                        # layer start: the first NB supers' non-last-chunk
                        # gathers first (their tables are long done —
                        # maximum Q7 cover), THEN the deferred
                        # last-quarter-AG trigger, THEN the chunk-3 gathers
                        # that need it
                        gs_batch = [alloc_gs12(L, b) for b in range(NB)]
                        for b in range(NB):
                            issue12(L, b, gs_batch[b], skip=(NCHUNK - 1,))
                        ag(L, NCHUNK - 1)
                        for b in range(NB):
                            issue12(L, b, gs_batch[b], only=(NCHUNK - 1,))
                        gs_cur = gs_batch[0]
                    elif s < NB:
                        gs_cur = gs_batch[s]
                    else:
                        gs_cur = alloc_gs12(L, s)
                        issue12(L, s, gs_cur)
                    agg(L, s, gs_cur)
                    if L == 1:
                        proj4(2, sup_ws[s][0], len(sup_ws[s]))
                        while nagq < NCHUNK - 1 and s >= agsup[nagq]:
                            ag(2, nagq)
                            nagq += 1

    nc.compile()
    return nc


def _in_map(consts, percore, c):
    return {
        "tab0": consts["tab0"], "tab1": consts["tab1"],
        "tab2": consts["tab2"], "tab3": consts["tab3"],
        "w1": consts["w1"], "w2": consts["w2"], "w3p": consts["w3p"],
        "nsdp": percore["nsdp"][c], "ndp": percore["ndp"][c],
        "sidx": percore["sidx"][c], "dcol": percore["dcol"][c],
        "sidx0": percore["sidx0"][c], "dcol0": percore["dcol0"][c],
        "iota": consts["iota"],
    }


def kernel(feat, W1, b1, W2, b2, W3, b3, src, dst):
    sched, consts, percore = _host_prep(feat, W1, b1, W2, b2, W3, b3, src, dst)
    nc = _build(sched)
    in_maps = [_in_map(consts, percore, c) for c in range(NCORE)]
    res = run_bass_kernel_spmd(nc, in_maps, core_ids=list(range(NCORE)))
    out = np.concatenate([res.results[c]["out"][:NP] for c in range(NCORE)],
                         axis=0)
    return np.ascontiguousarray(out.astype(np.float32))


# revision 41
# speedup vs baseline: 1.0129x; 1.0129x over previous
"""3-layer GCN (DGL GraphConv, norm='both') on 8 Trainium2 NeuronCores.

v6: L0 push-down projection + descriptor-minimized SPMD single-NEFF design.
  - L0 gathers RAW ns-prescaled features (512B rows, bf16) from four
    pre-staged replicated DRAM tables (global node quarters, int16-indexable)
    so gathers saturate the (serialized) Q7 SWDGE pipeline from t~=0 with no
    projection/AllGather gating. Aggregation runs at 256 feats (two matmul
    halves sharing each one-hot), then W1-projection + relu per dst window.
  - L1/L2: project own nodes on PE (bf16), quarter-AllGathers of the
    projected shard, per-edge SWDGE dma_gather (<=1024 idxs/call - HW limit),
    one-hot matmul segment-sum over 128-dst windows.
  - Per layer the in-degree norm is deferred into the next projection's
    per-node scale (nsd = ns*nd); L0's table carries ns only.
  - The last quarter-AG of layers 1-2 is deferred into the next layer's
    first super-group (fired just before its first chunk-3 gather call) so a
    pending AG trigger never head-blocks ready gather work on gpsimd.
  - Host (numpy) does index-only prep + input layout (bf16 cast, transpose,
    ns prescale) - SPMD needs identical instruction streams on all 8 cores.
"""

import numpy as np
import ml_dtypes

import concourse.bacc as bacc
import concourse.bass as bass
import concourse.mybir as mybir
import concourse.tile as tile
from concourse.bass import AP
from concourse.bass_utils import run_bass_kernel_spmd

BF16 = ml_dtypes.bfloat16
F32 = np.float32

N_NODES = 100000
D_IN, D_H1, D_H2, D_OUT = 256, 128, 128, 64
NCORE = 8
NP = N_NODES // NCORE        # 12500 nodes per core
NWIN = (NP + 127) // 128     # 98 windows (last holds 84)
NPPAD = NWIN * 128           # 12544
NCHUNK = 4
# L1/L2 src chunks (core-quartered AllGather tables): the LAST quarter-AG
# gates each layer boundary, keep it small (measured better than packing
# groups under k*1024). max sloc = 8*4064-1 < 32768.
QROWS_L = [4064, 4064, 3328, NP - 11456]
QCUM = [0]
for _q in QROWS_L:
    QCUM.append(QCUM[-1] + _q)
CHUNK_L = [q * NCORE for q in QROWS_L]           # gather chunk sizes
# L0 src chunks (global node quarters of the static feature table); sized so
# most (super0, chunk) groups fit one <=1024-idx call. max sloc0 = 32766.
Q0ROWS = [22411, 22411, 22411, 32767]
Q0CUM = [0]
for _q in Q0ROWS:
    Q0CUM.append(Q0CUM[-1] + _q)
GCALL = 1024                 # idxs per dma_gather call (hard HW limit)
SUPW = 4                     # windows per gather super-group (L1/L2)
SUPW0 = 2                    # windows per super-group (L0; 512B rows => 2x
#                              SBUF per tile, so half the windows)
NQUEUE = 4                   # SWDGE queues
# last proj window needed by quarter-AG q (L1/L2)
AGWIN = [((QCUM[q + 1] + 127) // 128) - 1 for q in range(NCHUNK)]


def _mk_sched(core, win, sloc, chunk, dcol, supw, zero_virgin_tails=False):
    """Static gather/aggregation schedule for one edge->(cell) bucketing.

    cells (dst-window, src-chunk) sized to the cross-core max count and
    concatenated per (super-group, chunk) with only the group tail padded
    to a 128-slot tile. Tiles may span window boundaries; each
    (window, chunk, tile) matmul gets its own one-hot column block with -1
    (no match) marking rows of other windows.
    """
    ncell = NCORE * NWIN * NCHUNK
    cell = ((core * NWIN + win) * NCHUNK + chunk).astype(np.int64)
    order = np.argsort(cell * 32768 + sloc, kind="stable")
    counts = np.bincount(cell, minlength=ncell).reshape(NCORE, NWIN, NCHUNK)
    m_wj = counts.max(axis=0).astype(np.int64)      # uniform cell sizes

    starts = np.zeros(ncell + 1, np.int64)
    np.cumsum(np.bincount(cell, minlength=ncell), out=starts[1:])

    nsup = (NWIN + supw - 1) // supw
    sup_ws = [list(range(s * supw, min((s + 1) * supw, NWIN)))
              for s in range(nsup)]

    A_wj = np.zeros((NWIN, NCHUNK), np.int64)   # absolute slot offset of cell
    gtile0 = {}
    gtiles = {}
    sup_tile0 = []
    p = 0
    for s in range(nsup):
        sup_tile0.append(p // 128)
        for j in range(NCHUNK):
            gtile0[(s, j)] = p // 128
            g0 = p
            for w in sup_ws[s]:
                A_wj[w, j] = p
                p += int(m_wj[w, j])
            p = ((p + 127) // 128) * 128
            gtiles[(s, j)] = (p - g0) // 128
    NT2 = p // 128
    sup_tile0.append(NT2)

    mm_of_sup = []
    nmm_w = np.zeros(NWIN, np.int64)
    for s in range(nsup):
        lst = []
        for w in sup_ws[s]:
            for j in range(NCHUNK):
                if m_wj[w, j] == 0:
                    continue
                a, m = int(A_wj[w, j]), int(m_wj[w, j])
                for t in range(a // 128, (a + m + 127) // 128):
                    lst.append((w, j, t))
                    nmm_w[w] += 1
        mm_of_sup.append(lst)
    NMM = int(sum(len(x) for x in mm_of_sup))
    max_sup_mm = max(len(x) for x in mm_of_sup)
    max_sup_tiles = max(sup_tile0[s + 1] - sup_tile0[s] for s in range(nsup))

    sidx = np.full((NCORE, NT2 * 128), -1, np.int16)
    dcol_slot = np.full((NCORE, NT2 * 128), -1, np.int64)
    for c in range(NCORE):
        for w in range(NWIN):
            for j in range(NCHUNK):
                cid = (c * NWIN + w) * NCHUNK + j
                e = order[starts[cid]:starts[cid + 1]]
                n = len(e)
                a = int(A_wj[w, j])
                sidx[c, a:a + n] = sloc[e]
                sidx[c, a + n:a + int(m_wj[w, j])] = 0   # interior pad
                dcol_slot[c, a:a + n] = dcol[e]
    # group tails keep idx=-1 (stripped at call end by the Q7) EXCEPT for
    # the first buffer rotation of the pool that first touches virgin SBUF:
    # there the tails gather row 0 (finite data) so no memset is needed to
    # keep stale-NaN out of the one-hot-zero multiplies
    if zero_virgin_tails:
        for s in range(min(3, nsup)):
            for j in range(NCHUNK):
                a0 = gtile0[(s, j)] * 128
                sz = gtiles[(s, j)] * 128
                seg = sidx[:, a0:a0 + sz]
                np.maximum(seg, 0, out=seg)

    dcol_mm = np.full((NCORE, NMM, 128), -1.0, F32)
    mm_idx = 0
    for s in range(nsup):
        for (w, j, t) in mm_of_sup[s]:
            a, m = int(A_wj[w, j]), int(m_wj[w, j])
            lo = max(a, t * 128)
            hi = min(a + m, (t + 1) * 128)
            for c in range(NCORE):
                seg = dcol_slot[c, lo:hi]
                dst_rows = np.arange(lo - t * 128, hi - t * 128)
                valid = seg >= 0
                dcol_mm[c, mm_idx, dst_rows[valid]] = seg[valid]
            mm_idx += 1
    assert mm_idx == NMM

    calls = []   # (chunk j, abs slot offset, n_idxs, super)
    for s in range(nsup):
        for j in range(NCHUNK):
            t0 = gtile0[(s, j)]
            off = t0 * 128
            greal = int(sum(m_wj[w, j] for w in sup_ws[s]))
            if zero_virgin_tails and s < 3:
                greal = gtiles[(s, j)] * 128   # cover tail slots too
            q = 0
            while q < greal:
                n = min(GCALL, ((greal - q + 15) // 16) * 16)
                calls.append((j, off + q, n, s))
                q += n

    def idx_layout(a):      # [n] int16 -> [128, n//16]
        return np.tile(a.reshape(-1, 16).T, (8, 1))

    sidx_l = np.stack([idx_layout(sidx[c]) for c in range(NCORE)])
    dcol_l = np.ascontiguousarray(
        dcol_mm.transpose(0, 2, 1)).astype(BF16)    # [NCORE, 128, NMM]

    return dict(NT2=NT2, NMM=NMM, calls=calls, mm_of_sup=mm_of_sup,
                nmm_w=nmm_w, sup_tile0=sup_tile0, nsup=nsup, sup_ws=sup_ws,
                max_sup_tiles=max_sup_tiles, max_sup_mm=max_sup_mm,
                sidx=sidx_l, dcol=dcol_l)


def _host_prep(feat, W1, b1, W2, b2, W3, b3, src, dst):
    src = np.asarray(src).astype(np.int64)
    dst = np.asarray(dst).astype(np.int64)
    for b in (b1, b2, b3):
        assert np.max(np.abs(np.asarray(b))) == 0.0, \
            "nonzero bias needs the undeferred-nd path"

    deg_out = np.bincount(src, minlength=N_NODES).astype(F32)
    deg_in = np.bincount(dst, minlength=N_NODES).astype(F32)
    ns = 1.0 / np.sqrt(np.maximum(deg_out, 1.0))
    nd = 1.0 / np.sqrt(np.maximum(deg_in, 1.0))
    nsd = ns * nd

    core = dst // NP
    dloc = dst % NP
    win = dloc // 128
    dcol = (dloc % 128).astype(np.int32)

    # L1/L2 bucketing: src by core-quarter of the AllGather table
    c_src = src // NP
    r_src = src % NP
    chunk = np.searchsorted(np.array(QCUM[1:]), r_src, side="right")
    qr = np.array([QROWS_L[q] for q in chunk])
    q0 = np.array([QCUM[q] for q in chunk])
    sloc = (c_src * qr + (r_src - q0)).astype(np.int16)
    s12 = _mk_sched(core, win, sloc, chunk, dcol, SUPW)

    # L0 bucketing: src by global quarter of the static feature table
    chunk0 = np.searchsorted(np.array(Q0CUM[1:]), src, side="right")
    b0 = np.array([Q0CUM[q] for q in chunk0])
    sloc0 = (src - b0).astype(np.int16)
    s0 = _mk_sched(core, win, sloc0, chunk0, dcol, SUPW0,
                   zero_virgin_tails=True)

    # L0 tables: ns-prescaled raw features, bf16, global node order,
    # quartered; replicated to every core.
    feat_pre = (np.asarray(feat).astype(F32) * ns[:, None]).astype(BF16)
    tabs = [np.ascontiguousarray(feat_pre[Q0CUM[j]:Q0CUM[j + 1]])
            for j in range(NCHUNK)]

    nsdp = np.zeros((NCORE, 128, NWIN), F32)
    ndp = np.zeros((NCORE, 128, NWIN), F32)
    for c in range(NCORE):
        for arr, dstp in ((nsd, nsdp), (nd, ndp)):
            v = np.zeros(NPPAD, F32)
            v[:NP] = arr[c * NP:(c + 1) * NP]
            dstp[c] = v.reshape(NWIN, 128).T

    ohchunk = min(32, max(s12["max_sup_mm"], s0["max_sup_mm"]))
    consts = dict(
        w1=np.asarray(W1).astype(F32).astype(BF16),
        w2=np.asarray(W2).astype(F32).astype(BF16),
        w3p=np.pad(np.asarray(W3).astype(F32), ((0, 0), (0, 128 - D_OUT))).astype(BF16),
        iota=np.tile(np.arange(128, dtype=F32).astype(BF16)[None, :],
                     (128, ohchunk)),
        tab0=tabs[0], tab1=tabs[1], tab2=tabs[2], tab3=tabs[3],
    )
    sched = dict(s12=s12, s0=s0, ohchunk=ohchunk)
    percore = dict(nsdp=nsdp, ndp=ndp,
                   sidx=s12["sidx"], dcol=s12["dcol"],
                   sidx0=s0["sidx"], dcol0=s0["dcol"])
    return sched, consts, percore


def _build(sched):
    s12 = sched["s12"]; s0 = sched["s0"]; ohchunk = sched["ohchunk"]
    NT2 = s12["NT2"]; NMM = s12["NMM"]
    NT20 = s0["NT2"]; NMM0 = s0["NMM"]
    nsup = s12["nsup"]; nsup0 = s0["nsup"]
    sup_ws = s12["sup_ws"]; sup_ws0 = s0["sup_ws"]
    # shared gs/oh pools: byte-identical padded slots for both layouts
    mst12 = s12["max_sup_tiles"]; mst0 = s0["max_sup_tiles"]
    GSB = max(mst12 * 256, mst0 * 512)          # bytes per partition
    GS12PAD = (GSB + 255) // 256
    GS0PAD = (GSB + 511) // 512
    msm = max(s12["max_sup_mm"], s0["max_sup_mm"])

    calls_by_sup = {}
    for c in s12["calls"]:
        calls_by_sup.setdefault(c[3], []).append(c)
    calls_by_sup0 = {}
    for c in s0["calls"]:
        calls_by_sup0.setdefault(c[3], []).append(c)

    nc = bacc.Bacc("TRN2", target_bir_lowering=False, debug=False,
                   num_devices=NCORE, num_swdge_queues=NQUEUE)
    dt = mybir.dt

    tab_t = [nc.dram_tensor(f"tab{j}", [Q0ROWS[j], D_IN], dt.bfloat16,
                            kind="ExternalInput") for j in range(NCHUNK)]
    w1_t = nc.dram_tensor("w1", [D_IN, D_H1], dt.bfloat16, kind="ExternalInput")
    w2_t = nc.dram_tensor("w2", [D_H1, D_H2], dt.bfloat16, kind="ExternalInput")
    w3_t = nc.dram_tensor("w3p", [D_H2, 128], dt.bfloat16, kind="ExternalInput")
    nsd_t = nc.dram_tensor("nsdp", [128, NWIN], dt.float32, kind="ExternalInput")
    nd_t = nc.dram_tensor("ndp", [128, NWIN], dt.float32, kind="ExternalInput")
    sidx_t = nc.dram_tensor("sidx", [128, NT2 * 8], dt.int16, kind="ExternalInput")
    dcol_t = nc.dram_tensor("dcol", [128, NMM], dt.bfloat16, kind="ExternalInput")
    sidx0_t = nc.dram_tensor("sidx0", [128, NT20 * 8], dt.int16,
                             kind="ExternalInput")
    dcol0_t = nc.dram_tensor("dcol0", [128, NMM0], dt.bfloat16,
                             kind="ExternalInput")
    iota_t = nc.dram_tensor("iota", [128, ohchunk * 128], dt.bfloat16,
                            kind="ExternalInput")
    out_t = nc.dram_tensor("out", [NP, D_OUT], dt.float32, kind="ExternalOutput")

    qcount = [0]

    def next_queue():
        q = qcount[0] % NQUEUE
        qcount[0] += 1
        return q

    with tile.TileContext(nc) as tc:
        with (
            tc.tile_pool(name="const", bufs=1) as cpool,
            tc.tile_pool(name="hbuf", bufs=1) as hpool,
            tc.tile_pool(name="gb", bufs=3) as gpool,
            tc.tile_pool(name="sx", bufs=3) as sxpool,
            tc.tile_pool(name="work", bufs=3) as wpool,
            tc.tile_pool(name="oh", bufs=3) as ohpool,
            tc.tile_pool(name="ps", bufs=6, space="PSUM") as ppool,
            tc.tile_pool(name="pj", bufs=2, space="PSUM") as pjpool,
            tc.tile_pool(name="dram", bufs=1, space="DRAM") as dpool,
        ):
            w1a_s = cpool.tile([128, D_H1], dt.bfloat16)
            w1b_s = cpool.tile([128, D_H1], dt.bfloat16)
            w2_s = cpool.tile([D_H1, D_H2], dt.bfloat16)
            w3_s = cpool.tile([D_H2, 128], dt.bfloat16)
            nsd_s = cpool.tile([128, NWIN], dt.float32)
            nd_s = cpool.tile([128, NWIN], dt.float32)
            sidx_s = cpool.tile([128, NT2 * 8], dt.int16)
            dcol_s = cpool.tile([128, NMM], dt.bfloat16)
            iota_s = cpool.tile([128, ohchunk * 128], dt.bfloat16)

            # iota/weights/norms on the sync queue (small, needed early);
            # the BIG L1/L2 index consts go on the Activation HWDGE queue so
            # they don't delay the streamed per-super L0 index slices that
            # the sync queue serves during startup
            nc.sync.dma_start(iota_s[:], iota_t.ap())
            nc.sync.dma_start(w1a_s[:], w1_t.ap()[0:128, :])
            nc.sync.dma_start(w1b_s[:], w1_t.ap()[128:256, :])
            nc.sync.dma_start(w2_s[:], w2_t.ap())
            nc.sync.dma_start(w3_s[:], w3_t.ap())
            nc.sync.dma_start(nsd_s[:], nsd_t.ap())
            nc.sync.dma_start(nd_s[:], nd_t.ap())
            nc.scalar.dma_start(sidx_s[:], sidx_t.ap())
            nc.scalar.dma_start(dcol_s[:], dcol_t.ap())

            h_s = hpool.tile([128, NWIN * 128], dt.bfloat16)   # hT (feat x nodes)

            tins = [dpool.tile([NP, 128], dt.bfloat16, name=f"tin{L}")
                    for L in (1, 2)]
            tfulls = [[dpool.tile([CHUNK_L[q], 128], dt.bfloat16,
                                  name=f"tfull{L}_{q}", addr_space="Shared")
                       for q in range(NCHUNK)] for L in (1, 2)]

            # zero the gather buffers once (stale bytes multiply with
            # one-hot zeros, so they must be finite). The first rotation
            # (L0 supers 0-2, zero_virgin_tails) gathers its full padded
            # extent, so only the bytes BEYOND each super's coverage need
            # zeroing — a few tiles instead of ~10us each on DVE ahead of
            # the first one-hot builds.
            for b in range(3):
                gz = gpool.tile([128, GS12PAD, 128], dt.bfloat16,
                                name="gsz", tag="gs",
                                padded_shape=[128, GS12PAD, 128])
                cov_tiles = 0
                if b < nsup0:
                    st = s0["sup_tile0"][b + 1] - s0["sup_tile0"][b]
                    cov_tiles = st * 2          # 512B gs0 tiles in 256B units
                if cov_tiles < GS12PAD:
                    nc.vector.memset(gz[:, cov_tiles:GS12PAD, :], 0.0)

            def proj4(L, w0, wn):
                # wn (<=4) projection windows into one PSUM bank, one wide
                # scale instr, per-window tin writes. L in {1, 2}; the
                # in-degree norm of layer L-1 is folded into nsd.
                ppj = pjpool.tile([128, wn * 128], dt.float32, name=f"pj{L}",
                                  tag="pj", padded_shape=[128, 512])
                for k in range(wn):
                    w = w0 + k
                    osl = ppj[:, k * 128:(k + 1) * 128]
                    rhs = w2_s if L == 1 else w3_s
                    nc.tensor.matmul(osl,
                                     lhsT=h_s[:, w * 128:(w + 1) * 128],
                                     rhs=rhs[:], start=True, stop=True)
                ssl = nsd_s[:, w0:w0 + wn]
                sbc = AP(ssl.tensor, ssl.offset, list(ssl.ap) + [[0, 128]])
                pbf = wpool.tile([128, wn * 128], dt.bfloat16, name="pbf",
                                 tag="pbf", padded_shape=[128, 512])
                nc.vector.tensor_tensor(out=pbf[:, 0:wn * 128],
                                        in0=ppj[:, 0:wn * 128], in1=sbc,
                                        op=mybir.AluOpType.mult)
                for k in range(wn):
                    w = w0 + k
                    wsz = min(128, NP - w * 128)
                    nc.sync.dma_start(tins[L - 1][w * 128:w * 128 + wsz, :],
                                      pbf[:wsz, k * 128:(k + 1) * 128])

            def ag(L, q):
                nc.gpsimd.collective_compute(
                    "AllGather", mybir.AluOpType.bypass,
                    replica_groups=[list(range(NCORE))],
                    ins=[tins[L - 1][QCUM[q]:QCUM[q + 1], :].opt()],
                    outs=[tfulls[L - 1][q][:].opt()],
                )

            def build_oh(dsl_src, mm0, nmm_s, tagsfx=""):
                oh = ohpool.tile([128, nmm_s * 128], dt.bfloat16,
                                 name=f"oh{tagsfx}", tag="oh",
                                 padded_shape=[128, msm * 128])
                q = 0
                while q < nmm_s:
                    nb = min(ohchunk, nmm_s - q)
                    dsl = dsl_src(q, nb)
                    bcast = AP(dsl.tensor, dsl.offset,
                               list(dsl.ap) + [[0, 128]])
                    nc.vector.tensor_tensor(
                        out=oh[:, q * 128:(q + nb) * 128],
                        in0=iota_s[:, 0:nb * 128],
                        in1=bcast,
                        op=mybir.AluOpType.is_equal)
                    q += nb
                return oh

            def agg0(s):
                # L0: gather 256-feat raw rows, aggregate both halves via
                # shared one-hots, project through W1 + relu into h_s
                stile0 = s0["sup_tile0"][s]
                stiles = s0["sup_tile0"][s + 1] - stile0
                smms = s0["mm_of_sup"][s]
                nmm_s = len(smms)
                mm0 = sum(len(s0["mm_of_sup"][ss]) for ss in range(s))
                sxs = sxpool.tile([128, stiles * 8], dt.int16, name="sx0",
                                  tag="sx0", padded_shape=[128, mst0 * 8])
                nc.sync.dma_start(
                    sxs[:], sidx0_t.ap()[:, stile0 * 8:(stile0 + stiles) * 8])
                dcs = sxpool.tile([128, nmm_s], dt.bfloat16, name="dc0",
                                  tag="dc0", padded_shape=[128, msm])
                nc.sync.dma_start(dcs[:], dcol0_t.ap()[:, mm0:mm0 + nmm_s])
                gs = gpool.tile([128, stiles, 256], dt.bfloat16,
                                name=f"gs0_{s}", tag="gs",
                                padded_shape=[128, GS0PAD, 256])
                for (j, off, n, cs) in calls_by_sup0.get(s, []):
                    rel = off // 128 - stile0
                    nc.gpsimd.dma_gather(
                        gs[:, rel:rel + (n + 127) // 128, :],
                        tab_d[j][:],
                        sxs[:, (off - stile0 * 128) // 16:
                            (off - stile0 * 128 + n) // 16],
                        n, n, 256,
                        queue_num=next_queue(),
                    )
                oh = build_oh(lambda q, nb: dcs[:, q:q + nb], 0, nmm_s, "0")
                apsA = {}
                apsB = {}
                done = {}
                for mi, (w, j, t) in enumerate(smms):
                    if w not in apsA:
                        apsA[w] = ppool.tile([128, 128], dt.float32,
                                             name="apA", tag="pp")
                        apsB[w] = ppool.tile([128, 128], dt.float32,
                                             name="apB", tag="pp")
                        done[w] = 0
                    k = done[w]
                    ohsl = oh[:, mi * 128:(mi + 1) * 128]
                    first = k == 0
                    last = k == int(s0["nmm_w"][w]) - 1
                    trel = t - stile0
                    nc.tensor.matmul(apsA[w][:], lhsT=gs[:, trel, 0:128],
                                     rhs=ohsl, start=first, stop=last)
                    nc.tensor.matmul(apsB[w][:], lhsT=gs[:, trel, 128:256],
                                     rhs=ohsl, start=first, stop=last)
                    done[w] = k + 1
                    if not last:
                        continue
                    evA = wpool.tile([128, 128], dt.bfloat16, name="evA",
                                     tag="evA")
                    evB = wpool.tile([128, 128], dt.bfloat16, name="evB",
                                     tag="evB")
                    nc.scalar.activation(evA[:], apsA[w][:],
                                         mybir.ActivationFunctionType.Copy)
                    nc.scalar.activation(evB[:], apsB[w][:],
                                         mybir.ActivationFunctionType.Copy)
                    hw = ppool.tile([128, 128], dt.float32, name="hw",
                                    tag="pp")
                    nc.tensor.matmul(hw[:], lhsT=w1a_s[:], rhs=evA[:],
                                     start=True, stop=False)
                    nc.tensor.matmul(hw[:], lhsT=w1b_s[:], rhs=evB[:],
                                     start=False, stop=True)
                    nc.scalar.activation(
                        h_s[:, w * 128:(w + 1) * 128], hw[:],
                        mybir.ActivationFunctionType.Relu)

            def alloc_gs12(L, s):
                stiles = s12["sup_tile0"][s + 1] - s12["sup_tile0"][s]
                return gpool.tile([128, stiles, 128], dt.bfloat16,
                                  name=f"gs{L}_{s}", tag="gs",
                                  padded_shape=[128, GS12PAD, 128])

            def issue12(L, s, gs, only=None, skip=None):
                stile0 = s12["sup_tile0"][s]
                for (j, off, n, cs) in calls_by_sup.get(s, []):
                    if only is not None and j not in only:
                        continue
                    if skip is not None and j in skip:
                        continue
                    rel = off // 128 - stile0
                    nc.gpsimd.dma_gather(
                        gs[:, rel:rel + (n + 127) // 128, :],
                        tfulls[L - 1][j][:],
                        sidx_s[:, off // 16:(off + n) // 16],
                        n, n, 128,
                        queue_num=next_queue(),
                    )

            def agg(L, s, gs):
                # L in {1, 2}: 128-feat aggregation from AllGather tables
                stile0 = s12["sup_tile0"][s]
                smms = s12["mm_of_sup"][s]
                nmm_s = len(smms)
                mm0 = sum(len(s12["mm_of_sup"][ss]) for ss in range(s))
                oh = build_oh(lambda q, nb: dcol_s[:, mm0 + q:mm0 + q + nb],
                              mm0, nmm_s)
                aps_of_w = {}
                done_of_w = {}
                for mi, (w, j, t) in enumerate(smms):
                    if w not in aps_of_w:
                        aps_of_w[w] = ppool.tile([128, 128], dt.float32,
                                                 name=f"ap{L}", tag="pp")
                        done_of_w[w] = 0
                    aps = aps_of_w[w]
                    k = done_of_w[w]
                    ohsl = oh[:, mi * 128:(mi + 1) * 128]
                    first, last = k == 0, k == int(s12["nmm_w"][w]) - 1
                    if L == 1:
                        nc.tensor.matmul(aps[:], lhsT=gs[:, t - stile0, :],
                                         rhs=ohsl, start=first, stop=last)
                    else:
                        nc.tensor.matmul(aps[:, 0:D_OUT], lhsT=ohsl,
                                         rhs=gs[:, t - stile0, 0:D_OUT],
                                         start=first, stop=last)
                    done_of_w[w] = k + 1
                    if not last:
                        continue
                    if L == 1:
                        nc.scalar.activation(
                            h_s[:, w * 128:(w + 1) * 128], aps[:],
                            mybir.ActivationFunctionType.Relu)
                    else:
                        wsz = min(128, NP - w * 128)
                        ob = wpool.tile([128, D_OUT], dt.float32, name="ob",
                                        tag="ob")
                        nc.vector.tensor_scalar(
                            out=ob[:], in0=aps[:, 0:D_OUT],
                            scalar1=nd_s[:, w:w + 1], scalar2=None,
                            op0=mybir.AluOpType.mult)
                        nc.sync.dma_start(
                            out_t.ap()[w * 128:w * 128 + wsz, :], ob[:wsz, :])

            # L0 tables to local DRAM (inputs land in DRAM already; use the
            # dram tensors directly as gather sources)
            tab_d = [tab_t[j].ap() for j in range(NCHUNK)]

            # ---- L0: gather raw features, aggregate, project; emit L1
            #      projections + quarter-AGs as windows complete ----
            agsup0 = [min(nsup0 - 1, (AGWIN[q] // SUPW0) + 2)
                      for q in range(NCHUNK)]
            nagq = 0
            for s in range(nsup0):
                agg0(s)
                proj4(1, sup_ws0[s][0], len(sup_ws0[s]))
                while nagq < NCHUNK - 1 and s >= agsup0[nagq]:
                    ag(1, nagq)
                    nagq += 1

            # ---- L1/L2 ----
            agsup = [min(nsup - 1, (AGWIN[q] // SUPW) + 2)
                     for q in range(NCHUNK)]
            for L in (1, 2):
                nagq = 0
                for s in range(nsup):
                    if s == 0:
                        # layer start: both supers' non-last-chunk gathers
                        # first (their tables are long done — maximum Q7
                        # cover), THEN the deferred last-quarter-AG trigger,
                        # THEN the chunk-3 gathers that need it
                        gs_a = alloc_gs12(L, 0)
                        gs_b = alloc_gs12(L, 1)
                        issue12(L, 0, gs_a, skip=(NCHUNK - 1,))
                        issue12(L, 1, gs_b, skip=(NCHUNK - 1,))
                        ag(L, NCHUNK - 1)
                        issue12(L, 0, gs_a, only=(NCHUNK - 1,))
                        issue12(L, 1, gs_b, only=(NCHUNK - 1,))
                        gs_cur = gs_a
                    elif s == 1:
                        gs_cur = gs_b
                    else:
                        gs_cur = alloc_gs12(L, s)
                        issue12(L, s, gs_cur)
                    agg(L, s, gs_cur)
                    if L == 1:
                        proj4(2, sup_ws[s][0], len(sup_ws[s]))
                        while nagq < NCHUNK - 1 and s >= agsup[nagq]:
                            ag(2, nagq)
                            nagq += 1

    nc.compile()
    return nc


def _in_map(consts, percore, c):
    return {
        "tab0": consts["tab0"], "tab1": consts["tab1"],
        "tab2": consts["tab2"], "tab3": consts["tab3"],
        "w1": consts["w1"], "w2": consts["w2"], "w3p": consts["w3p"],
        "nsdp": percore["nsdp"][c], "ndp": percore["ndp"][c],
        "sidx": percore["sidx"][c], "dcol": percore["dcol"][c],
        "sidx0": percore["sidx0"][c], "dcol0": percore["dcol0"][c],
        "iota": consts["iota"],
    }


def kernel(feat, W1, b1, W2, b2, W3, b3, src, dst):
    sched, consts, percore = _host_prep(feat, W1, b1, W2, b2, W3, b3, src, dst)
    nc = _build(sched)
    in_maps = [_in_map(consts, percore, c) for c in range(NCORE)]
    res = run_bass_kernel_spmd(nc, in_maps, core_ids=list(range(NCORE)))
    out = np.concatenate([res.results[c]["out"][:NP] for c in range(NCORE)],
                         axis=0)
    return np.ascontiguousarray(out.astype(np.float32))
